# revision 42
# baseline (speedup 1.0000x reference)
"""Trainium2 Bass kernel for nn_MAE_65025804861607 (MAE block: fused
qkv/dwconv/fc/depconv branch + 4-direction GroupMamba selective scan).

Data-parallel over batch: 16 images -> 8 cores x 2 images.

Per-core structure (one NeuronCore, 2 images):
  - The __reps timing loop is a hardware For_i loop, so the marginal cost
    of a rep measures actual execution, not NEFF-size overheads.
  - One [128, 4096] f32 PSUM tile spans all 8 banks; matmuls write
    bank-aligned 512-col slices and ACT/DVE consume multi-bank regions in
    single full-width ops.
  - conv branch: 1x1 convs + depthwise 3x3s collapse into host-fused
    shifted matmuls. conv1 pairs taps via a 1-column-shifted image copy in
    x_pad rows 64-127 (128-contraction, 6 matmuls/chunk); loops run
    tap-outer so consecutive matmuls share PE weights.
  - LayerNorm: per-pixel stats via PE column-sum matmuls; mu/rstd rows
    replicated by plain-slice log-doubling (stride-0 broadcast DMAs are
    3-8x slower per row); rstd folds into the projection rhs (xs).
  - selective scan: lanes are d-major (p = d*8+n) so dt/u replicate with
    one legal inner-broadcast DMA each and B/C with log-doubling; the DVE
    tensor_tensor_scan handles the 4 raster directions via reversed /
    pre-transposed operands; out-projection accumulates z chunks straight
    into PSUM across all (image, group) scans -- no z spill.
  - JSON post-passes: duplicate PE Ldweights removal (walrus ldw-opt is
    off here), wait subsumption, and >1-wait hoisting for walrus.
"""
import sys
import numpy as np

sys.path.insert(0, '/opt/trn_rl_repo')

import concourse.bass as bass
import concourse.mybir as mybir
from concourse.tile import TileContext
from concourse.bass_utils import run_bass_kernel_spmd

F32 = mybir.dt.float32
BF16 = mybir.dt.bfloat16
AF = mybir.ActivationFunctionType
OP = mybir.AluOpType

NCORES = 8
IPC = 2               # images per core
C = 64
H = W = 64
L = H * W             # 4096
NG, DG, DSTATE = 4, 16, 8
Hp, Wp = H + 2, W + 2
PADL = Hp * Wp        # 4356
TC = 512              # psum chunk = 8 image rows
NCH = L // TC         # 8

_CACHE = {}


# ----------------------------------------------------------------------------
# Walrus here allows only 1 embedded sem-wait per instruction (2 on
# EventSemaphore). Hoist excess waits into standalone EventSemaphores.
# ----------------------------------------------------------------------------
def _fix_waits_json(data):
    lim = {"EventSemaphore": 2}
    for fn in data.get("functions", []):
        for blk in fn.get("blocks", []):
            out = []
            for ins in blk.get("instructions", []):
                si = ins.get("sync_info")
                ow = (si or {}).get("on_wait") or []
                limit = lim.get(ins.get("opcode"), 1)
                if len(ow) > limit:
                    excess = ow[: len(ow) - limit]
                    si["on_wait"] = ow[len(ow) - limit:]
                    for k, wv in enumerate(excess):
                        out.append({
                            "debug": ins.get("debug", 0),
                            "engine": ins["engine"],
                            "ins": [], "outs": [],
                            "name": f"{ins['name']}_xw{k}",
                            "opcode": "EventSemaphore",
                            "sync_info": {"on_update": [], "on_wait": [wv]},
                        })
                out.append(ins)
            blk["instructions"] = out
    return data


# ----------------------------------------------------------------------------
# Drop PE weight reloads that are identical to the PE array's current
# contents (walrus' own ldw-opt is disabled in this toolchain). Matmults here
# are non-self-loading ("ldweights": false), so a Matmult after a dropped
# duplicate Ldweights simply reuses the already-loaded array. Any sem waits /
# updates on a dropped Ldweights move to the next PE instruction (later =
# conservative). State resets at block boundaries.
# ----------------------------------------------------------------------------
def _dedup_ldweights_json(data):
    import json as _json
    for fn in data.get("functions", []):
        for blk in fn.get("blocks", []):
            insts = blk.get("instructions", [])
            cur_sig = None
            pending = []   # sync entries to reattach to next PE instruction
            out = []
            for ins in insts:
                if ins.get("engine") != "PE":
                    out.append(ins)
                    continue
                op = ins.get("opcode")
                if op == "Ldweights":
                    sig = _json.dumps(
                        [ins.get("ins"), ins.get("tile_position"),
                         ins.get("tile_size"), ins.get("perf_mode"),
                         ins.get("is_transpose")],
                        sort_keys=True, default=str)
                    if sig == cur_sig:
                        si = ins.get("sync_info") or {}
                        pending.append((si.get("on_wait") or [],
                                        si.get("on_update") or []))
                        continue            # drop the duplicate
                    cur_sig = sig
                elif op == "Matmult":
                    if ins.get("ldweights"):
                        cur_sig = None      # self-loading: array changed
                else:
                    pass                    # other PE ops leave array intact
                if pending:
                    si = ins.setdefault("sync_info",
                                        {"on_wait": [], "on_update": []})
                    for ws, us in pending:
                        si["on_wait"] = (si.get("on_wait") or []) + ws
                        si["on_update"] = (si.get("on_update") or []) + us
                    pending = []
                out.append(ins)
            assert not pending, "dropped Ldweights at end of block"
            blk["instructions"] = out
    return data


# ----------------------------------------------------------------------------
# Drop semaphore waits already implied by an earlier wait on the same
# in-order queue (same engine+queue, same sem, >= an equal-or-higher value).
# Valid per block execution: sems only increment within a loop body
# (decrementing sems are excluded), and queues execute in order.
# ----------------------------------------------------------------------------
def _subsume_waits_json(data):
    for fn in data.get("functions", []):
        for blk in fn.get("blocks", []):
            insts = blk.get("instructions", [])
            dec_sems = set()
            for ins in insts:
                for u in (ins.get("sync_info") or {}).get("on_update") or []:
                    if (u.get("update_mode") != "sem-inc"
                            or u.get("update_value", 0) < 0):
                        dec_sems.add(u.get("id"))
            seen = {}
            for ins in insts:
                si = ins.get("sync_info")
                if not si or not si.get("on_wait"):
                    continue
                key = (ins.get("engine"), ins.get("queue"))
                hi = seen.setdefault(key, {})
                kept = []
                for w in si["on_wait"]:
                    if (w.get("sync_type") == "semaphore"
                            and w.get("wait_mode") == "sem-ge-imm"
                            and w.get("id") not in dec_sems):
                        m = hi.get(w["id"])
                        if m is not None and m >= w["wait_value"]:
                            continue
                        hi[w["id"]] = w["wait_value"]
                    kept.append(w)
                si["on_wait"] = kept
    return data


def _patch_bass_class():
    import json as _json
    cls = bass.Bass
    if getattr(cls, "_waitfix_patched", False):
        return
    orig = cls.to_json_bytes

    def patched(self, *a, **kw):
        data = _json.loads(orig(self, *a, **kw))
        _dedup_ldweights_json(data)
        _subsume_waits_json(data)
        _fix_waits_json(data)
        return _json.dumps(data).encode()

    cls.to_json_bytes = patched
    cls._waitfix_patched = True


# ----------------------------------------------------------------------------
# Host-side constant fusion
# ----------------------------------------------------------------------------
def _make_consts(inp):
    qkv_w = inp['qkv_w'][:, :, 0, 0, 0].astype(np.float64)      # (192, 64)
    dw_mid = inp['dw_w'][:, 0, 1, :, :].astype(np.float64)      # (192, 3, 3)
    fc_w = inp['fc_w'][:, :, 0, 0, 0].astype(np.float64)        # (9, 24)
    fc_b = inp['fc_b'].astype(np.float32)
    dep_mid = inp['dep_w'][:, :, 1, :, :].astype(np.float64)    # (64, 9, 3, 3)
    dep_b = inp['dep_b'].astype(np.float32)
    ln_g = inp['ln_g'].astype(np.float64)
    ln_b = inp['ln_b'].astype(np.float64)
    A = -np.exp(inp['A_log'].astype(np.float64))                # (NG, DG, DSTATE)
    Wdt, bdt = inp['Wdt'].astype(np.float64), inp['bdt'].astype(np.float64)
    WB, WC = inp['WB'].astype(np.float64), inp['WC'].astype(np.float64)
    Dp = inp['Dp'].astype(np.float64)
    out_w, out_b = inp['out_w'].astype(np.float64), inp['out_b'].astype(np.float64)

    c = {}
    # conv branch: fused tap weights Wt(ty,tx) = FCbd . diag(dw_tap) . Wqkv
    FCbd = np.zeros((72, 192))
    for d in range(8):
        for o in range(9):
            for k in range(24):
                FCbd[d * 9 + o, k * 8 + d] = fc_w[o, k]
    Wt = {}
    for ty in range(3):
        for tx in range(3):
            Wt[(ty, tx)] = FCbd @ (dw_mid[:, ty, tx][:, None] * qkv_w)  # (72,64)
    # paired taps (ty,0)+(ty,1) share one 128-contraction matmul; x_pad rows
    # 64-127 hold the 1-column-shifted image.
    wpair = np.zeros((128, 3 * 72), np.float32)
    wsing = np.zeros((64, 3 * 72), np.float32)
    for ty in range(3):
        wpair[0:64, 72 * ty:72 * ty + 72] = Wt[(ty, 0)].T
        wpair[64:128, 72 * ty:72 * ty + 72] = Wt[(ty, 1)].T
        wsing[:, 72 * ty:72 * ty + 72] = Wt[(ty, 2)].T
    c['wpair'] = wpair
    c['wsing'] = wsing
    f2b = np.zeros((72, 1), np.float32)
    for d in range(8):
        for o in range(9):
            f2b[d * 9 + o, 0] = fc_b[o]
    c['f2_bias'] = f2b
    bdep = np.zeros((72, 9 * 64), np.float32)
    for ty in range(3):
        for tx in range(3):
            k = ty * 3 + tx
            Bt = np.zeros((64, 72))
            for g in range(8):
                Bt[8 * g:8 * g + 8, 9 * g:9 * g + 9] = dep_mid[8 * g:8 * g + 8, :, ty, tx]
            bdep[:, 64 * k:64 * k + 64] = Bt.T
    c['bdep'] = bdep
    c['depb_pp'] = np.tile(dep_b, IPC).reshape(128, 1)

    # mamba projections (gamma folded into rows; mu handled via explicit
    # xcen). B/C interleaved per group: bc rows g*16+0..8 = B_g, +8..16 = C_g
    # so each (image, group) block of bc_sb is 16 contiguous rows.
    dtbc = np.zeros((64, 128))
    for g in range(NG):
        rows = slice(g * DG, (g + 1) * DG)
        gam = ln_g[rows][:, None]
        dtbc[rows, g * DG:(g + 1) * DG] = Wdt[g] * gam
        dtbc[rows, 64 + g * 16: 64 + g * 16 + 8] = WB[g] * gam
        dtbc[rows, 64 + g * 16 + 8: 64 + g * 16 + 16] = WC[g] * gam
    # duplicated halves: image i's matmul uses rows i*64.. so lhsT base
    # partition matches its rhs (xcen rows i*64..)
    c['dtbc_lhsT'] = np.vstack([dtbc, dtbc])
    # dt bias: bdt + Wdt^T beta (per group)
    bdte = np.zeros((64, 1))
    for g in range(NG):
        bet = ln_b[g * DG:(g + 1) * DG]
        bdte[g * DG:(g + 1) * DG, 0] = bdt[g] + Wdt[g].T @ bet
    c['bdt_pp'] = bdte
    fbc = np.zeros((64, 1), np.float32)
    for g in range(NG):
        bet = ln_b[g * DG:(g + 1) * DG]
        fbc[g * 16: g * 16 + 8, 0] = (WB[g].T @ bet).astype(np.float32)
        fbc[g * 16 + 8: g * 16 + 16, 0] = (WC[g].T @ bet).astype(np.float32)
    c['fbc_pp'] = fbc
    c['beta_pp'] = np.tile(ln_b, IPC).reshape(128, 1)
    c['gamma_pp'] = np.tile(ln_g, IPC).reshape(128, 1)
    # scan lanes are d-major: p = d*8 + n (dt replicates with one inner-
    # broadcast DMA, B/C with plain-slice log-doubling)
    app = np.zeros((128, NG), np.float32)
    for g in range(NG):
        for n in range(DSTATE):
            for d in range(DG):
                app[d * 8 + n, g] = A[g, d, n]
    c['a_pp'] = app
    opl = np.zeros((128, NG * 64), np.float32)
    for g in range(NG):
        for n in range(DSTATE):
            for d in range(DG):
                opl[d * 8 + n, g * 64:(g + 1) * 64] = out_w[:, g * DG + d]
    c['outproj_lhsT'] = opl
    dpf = out_w * Dp.reshape(-1)[None, :]
    dpl = np.zeros((128, 128), np.float32)
    for i in range(IPC):
        dpl[i * 64:(i + 1) * 64, i * 64:(i + 1) * 64] = dpf.T
    c['dp_lhsT'] = dpl
    outb_eff = out_b + dpf @ ln_b
    c['outb_pp'] = np.tile(outb_eff, IPC).reshape(128, 1)
    c['ca1_lhsT'] = (inp['ca_w1'].T / L).astype(np.float32)     # fold 1/L mean
    c['ca1_b'] = inp['ca_b1'].reshape(16, 1).astype(np.float32)
    c['ca2_lhsT'] = inp['ca_w2'].T.astype(np.float32)
    c['ones64'] = np.ones((64, 1), np.float32)
    c['ca2bn_pp'] = -np.tile(inp['ca_b2'], IPC).reshape(128, 1)
    sl = np.zeros((128, 2), np.float32)
    sl[0:64, 0] = 1.0
    sl[64:128, 1] = 1.0
    c['stats_lhsT'] = sl
    return {k: np.ascontiguousarray(np.asarray(v, np.float32)) for k, v in c.items()}


CONST_SPECS = [
    ('wpair', [128, 3 * 72], BF16), ('wsing', [64, 3 * 72], BF16),
    ('f2_bias', [72, 1], F32),
    ('bdep', [72, 9 * 64], BF16), ('depb_pp', [128, 1], F32),
    ('dtbc_lhsT', [128, 128], BF16),
    ('bdt_pp', [64, 1], F32), ('fbc_pp', [64, 1], F32),
    ('beta_pp', [128, 1], F32), ('gamma_pp', [128, 1], F32),
    ('a_pp', [128, NG], F32),
    ('outproj_lhsT', [128, NG * 64], BF16), ('dp_lhsT', [128, 128], BF16),
    ('outb_pp', [128, 1], F32),
    ('ca1_lhsT', [64, 16], BF16), ('ca1_b', [16, 1], F32),
    ('ca2_lhsT', [16, 64], BF16), ('ca2bn_pp', [128, 1], F32),
    ('ones64', [64, 1], F32),
    ('stats_lhsT', [128, 2], BF16),
]


def _build(reps=1):
    _patch_bass_class()
    nc = bass.Bass("TRN2")
    xin = nc.declare_dram_parameter("x", [IPC, C, H, W], F32, isOutput=False)
    out = nc.declare_dram_parameter("out", [IPC, C, H, W], F32, isOutput=True)
    dram = {n: nc.declare_dram_parameter(n, s, F32, isOutput=False)
            for n, s, _ in CONST_SPECS}

    xin_f = xin.rearrange("i c h w -> (i c) (h w)")
    out_f = out.rearrange("i c h w -> (i c) (h w)")

    with TileContext(nc) as tc:
        with tc.tile_pool(name="const", bufs=1) as kpool, \
             tc.tile_pool(name="pers", bufs=1) as pp, \
             tc.tile_pool(name="work", bufs=2) as wp, \
             tc.tile_pool(name="ps", bufs=1, space="PSUM") as ps:

            kt = {}
            for name, shape, dt in CONST_SPECS:
                kt[name] = kpool.tile(shape, dt, tag=name, name=name)
                eng = nc.gpsimd if dt == BF16 else nc.sync
                eng.dma_start(kt[name][:], dram[name][:])

            # ---- persistent SBUF tiles (stable addresses across reps;
            # written in place every rep, tags never shared) ----
            xb = pp.tile([128, L], BF16, tag="xb", name="xb")
            x_pad = [pp.tile([128, PADL], BF16, tag=f"x_pad{i}", name=f"x_pad{i}")
                     for i in range(IPC)]
            f2_pad = [pp.tile([72, PADL], BF16, tag=f"f2_pad{i}", name=f"f2_pad{i}")
                      for i in range(IPC)]
            srow = pp.tile([2, L], BF16, tag="srow", name="srow")
            xcen = pp.tile([128, L], BF16, tag="xcen", name="xcen")
            xs = pp.tile([128, L], BF16, tag="xs", name="xs")
            dt_hat = pp.tile([128, L], BF16, tag="dt_hat", name="dt_hat")
            bc_sb = pp.tile([128, L], BF16, tag="bc_sb", name="bc_sb")
            u_sb = pp.tile([128, L], BF16, tag="u_sb", name="u_sb")
            xnc = pp.tile([128, L], BF16, tag="xnc", name="xnc")
            y_sb = pp.tile([128, L], BF16, tag="y_sb", name="y_sb")
            oc_sb = pp.tile([128, L], BF16, tag="oc_sb", name="oc_sb")
            ymean = pp.tile([128, 1], F32, tag="ymean", name="ymean")
            ca_sb = pp.tile([128, 1], F32, tag="ca_sb", name="ca_sb")
            # the whole PSUM (8 banks) as one tile; matmuls write bank-aligned
            # slices, ACT/DVE consume multi-bank regions in one op
            psA = ps.tile([128, L], F32, tag="psA", name="psA")

            # pad borders zeroed once; interiors rewritten every rep and the
            # pad slots are never reused, so borders stay zero.
            for i in range(IPC):
                nc.vector.memset(x_pad[i][:], 0.0)
                nc.vector.memset(f2_pad[i][:], 0.0)

            from contextlib import nullcontext
            with tc.For_i(0, reps) if reps > 1 else nullcontext():
                # ---- input loads: one casting DMA (gpsimd), pads derive
                # from xb on the otherwise-idle sync/scalar queues ----
                nc.gpsimd.dma_start(xb[:], xin_f[:, :])
                for i, eng in ((0, nc.sync), (1, nc.scalar)):
                    xpv = x_pad[i].rearrange("c (h w) -> c h w", h=Hp)
                    src = xb[i * 64:(i + 1) * 64, :].rearrange(
                        "c (h w) -> c h w", h=H)
                    eng.dma_start(xpv[0:64, 1:H + 1, 1:W + 1], src)
                    eng.dma_start(xpv[64:128, 1:H + 1, 0:W], src)

                # ---- conv1: f2 = sum_tap Wt_tap @ x_shift_tap + fc_b ----
                # tap-outer, chunk-inner: consecutive matmuls share weights so
                # _dedup_ldweights_json keeps one PE array load per tap
                for i in range(IPC):
                    xpv = x_pad[i].rearrange("c (h w) -> c h w", h=Hp)
                    f2v = f2_pad[i].rearrange("c (h w) -> c h w", h=Hp)
                    for ty in range(3):
                        for cb in range(NCH):
                            nc.tensor.matmul(
                                psA[0:72, cb * TC:(cb + 1) * TC],
                                kt['wpair'][:, 72 * ty:72 * ty + 72],
                                xpv[:, 8 * cb + ty: 8 * cb + ty + 8, 0:64],
                                start=(ty == 0), stop=False,
                                skip_group_check=True)
                    for ty in range(3):
                        for cb in range(NCH):
                            nc.tensor.matmul(
                                psA[0:72, cb * TC:(cb + 1) * TC],
                                kt['wsing'][:, 72 * ty:72 * ty + 72],
                                xpv[0:64, 8 * cb + ty: 8 * cb + ty + 8, 2:66],
                                start=False, stop=(ty == 2),
                                skip_group_check=True)
                    nc.scalar.activation(
                        f2v[:, 1:H + 1, 1:W + 1],
                        psA[0:72, :].rearrange("c (a b) -> c a b", a=H),
                        AF.Identity, bias=kt['f2_bias'][:])

                # ---- stats: per-pixel mean/rstd over channels ----
                sqf = pp.tile([128, L], BF16, tag="dAc", name="sqf")
                nc.vector.tensor_mul(sqf[:], xb[:], xb[:])
                for cb in range(NCH):
                    cs = slice(cb * TC, (cb + 1) * TC)
                    nc.tensor.matmul(psA[0:2, cs], kt['stats_lhsT'][:],
                                     xb[:, cs], start=True, stop=True,
                                     skip_group_check=True)
                for cb in range(NCH):
                    cs = slice(cb * TC, (cb + 1) * TC)
                    nc.tensor.matmul(psA[32:34, cs], kt['stats_lhsT'][:],
                                     sqf[:, cs], start=True, stop=True,
                                     tile_position=(0, 32),
                                     skip_group_check=True)
                nc.scalar.activation(srow[0:2, :], psA[0:2, :], AF.Identity,
                                     scale=1.0 / 64)
                nc.scalar.activation(xcen[0:2, :], psA[32:34, :], AF.Identity,
                                     scale=1.0 / 64)
                # mr: cols 0:L = per-row mu, cols L:2L = per-row rstd;
                # replicated by log-doubling (stride-0 broadcast DMAs are
                # ~16x slower per row than real copies)
                mr = pp.tile([128, 2 * L], BF16, tag="bc_rep", name="mr",
                             bufs=2)
                for i in range(IPC):
                    nc.scalar.dma_start(mr[i * 64:i * 64 + 1, 0:L],
                                        srow[i:i + 1, :])
                nc.vector.tensor_mul(srow[0:2, :], srow[0:2, :], srow[0:2, :])
                nc.vector.tensor_sub(xcen[0:2, :], xcen[0:2, :], srow[0:2, :])
                # (the reference's +1e-5 eps is below bf16 resolution at var~1)
                nc.scalar.activation(xcen[0:2, :], xcen[0:2, :], AF.Ln)
                nc.scalar.activation(xcen[0:2, :], xcen[0:2, :], AF.Exp,
                                     scale=-0.5)
                for i in range(IPC):
                    nc.scalar.dma_start(mr[i * 64:i * 64 + 1, L:2 * L],
                                        xcen[i:i + 1, :])
                for k in (1, 2, 4, 8, 16, 32):
                    for i in range(IPC):
                        nc.scalar.dma_start(mr[i * 64 + k: i * 64 + 2 * k, :],
                                            mr[i * 64: i * 64 + k, :])
                musb = mr[:, 0:L]
                rstb = mr[:, L:2 * L]
                nc.vector.tensor_sub(xcen[:], xb[:], musb[:])
                # xs = rstd * (x - mu): fold the per-pixel rstd into the
                # projection rhs (linear), so no 128-row rstd broadcast needed
                nc.vector.tensor_mul(xs[:], xcen[:], rstb[:])

                # ---- dt/B/C projections (rhs pre-scaled by rstd) ----
                for i in range(IPC):
                    for cb in range(NCH):
                        cs = slice(cb * TC, (cb + 1) * TC)
                        nc.tensor.matmul(psA[:, cs],
                                         kt['dtbc_lhsT'][i * 64:(i + 1) * 64, :],
                                         xs[i * 64:(i + 1) * 64, cs],
                                         start=True, stop=True,
                                         skip_group_check=True)
                    et = pp.tile([128, L], BF16, tag="h_sb", name=f"et{i}")
                    nc.scalar.activation(et[0:64, :], psA[0:64, :], AF.Exp,
                                         bias=kt['bdt_pp'][:])
                    nc.scalar.activation(dt_hat[i * 64:(i + 1) * 64, :],
                                         et[0:64, :], AF.Ln,
                                         bias=kt['ones64'][:])
                    nc.scalar.activation(bc_sb[i * 64:(i + 1) * 64, :],
                                         psA[64:128, :], AF.Identity,
                                         bias=kt['fbc_pp'][:])
                # xn = gamma * xs ; u = dt * (xn + beta)
                nc.vector.tensor_scalar_mul(xnc[:], xs[:], kt['gamma_pp'][:])
                nc.vector.tensor_mul(u_sb[:], dt_hat[:], xnc[:])
                nc.vector.scalar_tensor_tensor(u_sb[:], dt_hat[:], kt['beta_pp'][:],
                                               u_sb[:], OP.mult, OP.add)

                # ---- conv2: out_conv = sum_tap BDdep_tap @ f2_shift_tap ----
                # tap-outer, chunk-inner (one weight load per tap per image)
                for i in range(IPC):
                    f2v = f2_pad[i].rearrange("c (h w) -> c h w", h=Hp)
                    for ty in range(3):
                        for tx in range(3):
                            k = ty * 3 + tx
                            for cb in range(NCH):
                                nc.tensor.matmul(
                                    psA[i * 64:(i + 1) * 64,
                                        cb * TC:(cb + 1) * TC],
                                    kt['bdep'][:, 64 * k:64 * k + 64],
                                    f2v[:, 8 * cb + ty: 8 * cb + ty + 8, tx: tx + 64],
                                    start=(k == 0), stop=(k == 8),
                                    tile_position=(0, i * 64),
                                    skip_group_check=True)
                nc.scalar.activation(oc_sb[:], psA[:], AF.Identity,
                                     bias=kt['depb_pp'][:])

                # ---- selective scans; out-proj accumulates across all of
                # psA (start on g==0 per image half, stop on the dp matmul) --
                for i in range(IPC):
                    for g in range(NG):
                        colmajor = g >= 2
                        rev = (g % 2 == 1)
                        rs = slice(i * 64 + g * 16, i * 64 + (g + 1) * 16)
                        # d-major lanes: dt/u replicate 8x consecutive via
                        # one inner-broadcast DMA each (parallel queues)
                        du = pp.tile([128, 2 * L], BF16, tag="du", name="du",
                                     bufs=2)
                        nc.sync.dma_start(
                            du[:, 0:L],
                            dt_hat[rs, :].unsqueeze(1).broadcast_to([16, 8, L]))
                        nc.gpsimd.dma_start(
                            du[:, L:2 * L],
                            u_sb[rs, :].unsqueeze(1).broadcast_to([16, 8, L]))
                        dt_rep = du[:, 0:L]
                        u_rep = du[:, L:2 * L]
                        # B/C (lane index n = p%8): plain-slice log-doubling
                        bc_rep = pp.tile([128, 2 * L], BF16, tag="bc_rep",
                                         name="bc_rep", bufs=2)
                        base = i * 64 + g * 16
                        nc.scalar.dma_start(bc_rep[0:8, 0:L],
                                            bc_sb[base: base + 8, :])
                        nc.scalar.dma_start(bc_rep[0:8, L:2 * L],
                                            bc_sb[base + 8: base + 16, :])
                        nc.scalar.dma_start(bc_rep[8:16, :], bc_rep[0:8, :])
                        nc.scalar.dma_start(bc_rep[16:32, :], bc_rep[0:16, :])
                        nc.scalar.dma_start(bc_rep[32:64, :], bc_rep[0:32, :])
                        nc.scalar.dma_start(bc_rep[64:128, :], bc_rep[0:64, :])
                        b_rep = bc_rep[:, 0:L]
                        c_rep = bc_rep[:, L:2 * L]
                        h_sb = pp.tile([128, L], BF16, tag="h_sb", name="h_sb")
                        # dA/dBx; column-major groups pre-transpose into
                        # dedicated slots (scan operands must be 2D)
                        if colmajor:
                            dA = pp.tile([128, L], BF16, tag="dAc", name="dA")
                            dBx = pp.tile([128, L], BF16, tag="dBc", name="dBx")
                            nc.scalar.activation(
                                dA.rearrange("p (x y) -> p y x", x=W),
                                dt_rep.rearrange("p (y x) -> p y x", y=H),
                                AF.Exp, scale=kt['a_pp'][:, g:g + 1])
                            nc.vector.tensor_tensor(
                                dBx.rearrange("p (x y) -> p y x", x=W),
                                u_rep.rearrange("p (y x) -> p y x", y=H),
                                b_rep.rearrange("p (y x) -> p y x", y=H),
                                OP.mult)
                        else:
                            dA, dBx = dt_rep, u_rep
                            nc.scalar.activation(dA[:], dt_rep[:], AF.Exp,
                                                 scale=kt['a_pp'][:, g:g + 1])
                            nc.vector.tensor_mul(dBx[:], u_rep[:], b_rep[:])
                        if rev:
                            nc.vector.tensor_tensor_scan(
                                h_sb[:, ::-1], dA[:, ::-1], dBx[:, ::-1], 0.0,
                                OP.mult, OP.add)
                        else:
                            nc.vector.tensor_tensor_scan(
                                h_sb[:], dA[:], dBx[:], 0.0, OP.mult, OP.add)
                        z = pp.tile([128, L], BF16, tag="du", name="z",
                                    bufs=2)
                        if colmajor:
                            nc.vector.tensor_tensor(
                                z.rearrange("p (y x) -> p y x", y=H),
                                h_sb.rearrange("p (x y) -> p y x", x=W),
                                c_rep.rearrange("p (y x) -> p y x", y=H),
                                OP.mult)
                        else:
                            nc.vector.tensor_mul(z[:], h_sb[:], c_rep[:])
                        for cb in range(NCH):
                            cs = slice(cb * TC, (cb + 1) * TC)
                            nc.tensor.matmul(
                                psA[i * 64:(i + 1) * 64, cs],
                                kt['outproj_lhsT'][:, g * 64:(g + 1) * 64],
                                z[:, cs], start=(g == 0), stop=False,
                                tile_position=(0, i * 64),
                                skip_group_check=True)
                for cb in range(NCH):
                    cs = slice(cb * TC, (cb + 1) * TC)
                    nc.tensor.matmul(psA[:, cs], kt['dp_lhsT'][:], xnc[:, cs],
                                     start=False, stop=True,
                                     skip_group_check=True)
                nc.scalar.activation(y_sb[:], psA[:], AF.Identity,
                                     bias=kt['outb_pp'][:],
                                     accum_out=ymean[:, 0:1])

                # ---- CA gate ----
                ymc = []
                for i in range(IPC):
                    t = wp.tile([64, 1], BF16, tag=f"ymc{i}", name=f"ymc{i}")
                    nc.gpsimd.dma_start(t[:], ymean[i * 64:(i + 1) * 64, :])
                    ymc.append(t)
                for i in range(IPC):
                    nc.tensor.matmul(psA[0:16, i:i + 1], kt['ca1_lhsT'][:],
                                     ymc[i][:], start=True, stop=True,
                                     skip_group_check=True)
                ca1s = wp.tile([16, IPC], BF16, tag="ca1s", name="ca1s")
                nc.scalar.activation(ca1s[:], psA[0:16, 0:IPC], AF.Relu,
                                     bias=kt['ca1_b'][:])
                for i in range(IPC):
                    nc.tensor.matmul(psA[i * 64:(i + 1) * 64, 4:5],
                                     kt['ca2_lhsT'][:],
                                     ca1s[:, i:i + 1], start=True, stop=True,
                                     tile_position=(0, i * 64),
                                     skip_group_check=True)
                nc.scalar.activation(ca_sb[:], psA[:, 4:5], AF.Exp, scale=-1.0,
                                     bias=kt['ca2bn_pp'][:])
                nc.vector.tensor_scalar_add(ca_sb[:], ca_sb[:], 1.0)
                nc.vector.reciprocal(ca_sb[:], ca_sb[:])

                # ---- final combine: out = x + oc(+depb) + ca*y ----
                # (u_sb and xcen are dead by now; reuse their APs in place)
                nc.vector.tensor_add(u_sb[:], oc_sb[:], xb[:])
                nc.vector.scalar_tensor_tensor(xcen[:], y_sb[:], ca_sb[:],
                                               u_sb[:], OP.mult, OP.add)
                nc.gpsimd.dma_start(out_f[:, :], xcen[:])

    # extended/loop InstISA instructions need .instr bytes populated before
    # the NEFF compiler sees them ("ISA wrong length" otherwise)
    mybir.codegen_inst_isa_subclasses(nc)
    return nc


def kernel(__reps=1, **inputs):
    inputs = {k: np.asarray(v) for k, v in inputs.items()}
    x = inputs['x'].astype(np.float32)
    key = f"v2r{__reps}"
    if key not in _CACHE:
        _CACHE[key] = _build(__reps)
    nc = _CACHE[key]
    consts = _make_consts(inputs)
    in_maps = []
    for core in range(NCORES):
        m = {'x': np.ascontiguousarray(x[core * IPC:(core + 1) * IPC])}
        m.update(consts)
        in_maps.append(m)
    res = run_bass_kernel_spmd(nc, in_maps, list(range(NCORES)))
    outs = [res.results[i]['out'] for i in range(NCORES)]
    return np.concatenate(outs, axis=0).astype(np.float32)


# revision 43
# speedup vs baseline: 1.1208x; 1.1208x over previous
"""Trainium2 Bass kernel for nn_MAE_65025804861607 (MAE block: fused
qkv/dwconv/fc/depconv branch + 4-direction GroupMamba selective scan).

Data-parallel over batch: 16 images -> 8 cores x 2 images.

Per-core structure (one NeuronCore, 2 images):
  - The __reps timing loop is a hardware For_i loop, so the marginal cost
    of a rep measures actual execution, not NEFF-size overheads.
  - One [128, 4096] f32 PSUM tile spans all 8 banks; matmuls write
    bank-aligned 512-col slices and ACT/DVE consume multi-bank regions in
    single full-width ops.
  - conv branch: 1x1 convs + depthwise 3x3s collapse into host-fused
    shifted matmuls. conv1 pairs taps via a 1-column-shifted image copy in
    x_pad rows 64-127 (128-contraction, 6 matmuls/chunk); loops run
    tap-outer so consecutive matmuls share PE weights.
  - LayerNorm: per-pixel stats via PE column-sum matmuls; mu/rstd rows
    replicated by plain-slice log-doubling (stride-0 broadcast DMAs are
    3-8x slower per row); rstd folds into the projection rhs (xs).
  - selective scan: lanes are d-major (p = d*8+n) so dt/u replicate with
    one legal inner-broadcast DMA each and B/C with log-doubling; the DVE
    tensor_tensor_scan handles the 4 raster directions via reversed /
    pre-transposed operands; out-projection accumulates z chunks straight
    into PSUM across all (image, group) scans -- no z spill.
  - JSON post-passes: duplicate PE Ldweights removal (walrus ldw-opt is
    off here), wait subsumption, and >1-wait hoisting for walrus.
"""
import sys
import numpy as np

sys.path.insert(0, '/opt/trn_rl_repo')

import concourse.bass as bass
import concourse.mybir as mybir
from concourse.tile import TileContext
from concourse.bass_utils import run_bass_kernel_spmd

F32 = mybir.dt.float32
BF16 = mybir.dt.bfloat16
AF = mybir.ActivationFunctionType
OP = mybir.AluOpType

NCORES = 8
IPC = 2               # images per core
C = 64
H = W = 64
L = H * W             # 4096
NG, DG, DSTATE = 4, 16, 8
Hp, Wp = H + 2, W + 2
PADL = Hp * Wp        # 4356
TC = 512              # psum chunk = 8 image rows
NCH = L // TC         # 8

_CACHE = {}


# ----------------------------------------------------------------------------
# Walrus here allows only 1 embedded sem-wait per instruction (2 on
# EventSemaphore). Hoist excess waits into standalone EventSemaphores.
# ----------------------------------------------------------------------------
def _fix_waits_json(data):
    lim = {"EventSemaphore": 2}
    for fn in data.get("functions", []):
        for blk in fn.get("blocks", []):
            out = []
            for ins in blk.get("instructions", []):
                si = ins.get("sync_info")
                ow = (si or {}).get("on_wait") or []
                limit = lim.get(ins.get("opcode"), 1)
                if len(ow) > limit:
                    excess = ow[: len(ow) - limit]
                    si["on_wait"] = ow[len(ow) - limit:]
                    for k, wv in enumerate(excess):
                        out.append({
                            "debug": ins.get("debug", 0),
                            "engine": ins["engine"],
                            "ins": [], "outs": [],
                            "name": f"{ins['name']}_xw{k}",
                            "opcode": "EventSemaphore",
                            "sync_info": {"on_update": [], "on_wait": [wv]},
                        })
                out.append(ins)
            blk["instructions"] = out
    return data


# ----------------------------------------------------------------------------
# Drop PE weight reloads that are identical to the PE array's current
# contents (walrus' own ldw-opt is disabled in this toolchain). Matmults here
# are non-self-loading ("ldweights": false), so a Matmult after a dropped
# duplicate Ldweights simply reuses the already-loaded array. Any sem waits /
# updates on a dropped Ldweights move to the next PE instruction (later =
# conservative). State resets at block boundaries.
# ----------------------------------------------------------------------------
def _dedup_ldweights_json(data):
    import json as _json
    for fn in data.get("functions", []):
        for blk in fn.get("blocks", []):
            insts = blk.get("instructions", [])
            cur_sig = None
            pending = []   # sync entries to reattach to next PE instruction
            out = []
            for ins in insts:
                if ins.get("engine") != "PE":
                    out.append(ins)
                    continue
                op = ins.get("opcode")
                if op == "Ldweights":
                    sig = _json.dumps(
                        [ins.get("ins"), ins.get("tile_position"),
                         ins.get("tile_size"), ins.get("perf_mode"),
                         ins.get("is_transpose")],
                        sort_keys=True, default=str)
                    if sig == cur_sig:
                        si = ins.get("sync_info") or {}
                        pending.append((si.get("on_wait") or [],
                                        si.get("on_update") or []))
                        continue            # drop the duplicate
                    cur_sig = sig
                elif op == "Matmult":
                    if ins.get("ldweights"):
                        cur_sig = None      # self-loading: array changed
                else:
                    pass                    # other PE ops leave array intact
                if pending:
                    si = ins.setdefault("sync_info",
                                        {"on_wait": [], "on_update": []})
                    for ws, us in pending:
                        si["on_wait"] = (si.get("on_wait") or []) + ws
                        si["on_update"] = (si.get("on_update") or []) + us
                    pending = []
                out.append(ins)
            assert not pending, "dropped Ldweights at end of block"
            blk["instructions"] = out
    return data


# ----------------------------------------------------------------------------
# Drop semaphore waits already implied by an earlier wait on the same
# in-order queue (same engine+queue, same sem, >= an equal-or-higher value).
# Valid per block execution: sems only increment within a loop body
# (decrementing sems are excluded), and queues execute in order.
# ----------------------------------------------------------------------------
def _subsume_waits_json(data):
    for fn in data.get("functions", []):
        for blk in fn.get("blocks", []):
            insts = blk.get("instructions", [])
            dec_sems = set()
            for ins in insts:
                for u in (ins.get("sync_info") or {}).get("on_update") or []:
                    if (u.get("update_mode") != "sem-inc"
                            or u.get("update_value", 0) < 0):
                        dec_sems.add(u.get("id"))
            seen = {}
            for ins in insts:
                si = ins.get("sync_info")
                if not si or not si.get("on_wait"):
                    continue
                key = (ins.get("engine"), ins.get("queue"))
                hi = seen.setdefault(key, {})
                kept = []
                for w in si["on_wait"]:
                    if (w.get("sync_type") == "semaphore"
                            and w.get("wait_mode") == "sem-ge-imm"
                            and w.get("id") not in dec_sems):
                        m = hi.get(w["id"])
                        if m is not None and m >= w["wait_value"]:
                            continue
                        hi[w["id"]] = w["wait_value"]
                    kept.append(w)
                si["on_wait"] = kept
    return data


def _patch_bass_class():
    import json as _json
    cls = bass.Bass
    if getattr(cls, "_waitfix_patched", False):
        return
    orig = cls.to_json_bytes

    def patched(self, *a, **kw):
        data = _json.loads(orig(self, *a, **kw))
        _dedup_ldweights_json(data)
        _subsume_waits_json(data)
        _fix_waits_json(data)
        return _json.dumps(data).encode()

    cls.to_json_bytes = patched
    cls._waitfix_patched = True


# ----------------------------------------------------------------------------
# Host-side constant fusion
# ----------------------------------------------------------------------------
def _make_consts(inp):
    qkv_w = inp['qkv_w'][:, :, 0, 0, 0].astype(np.float64)      # (192, 64)
    dw_mid = inp['dw_w'][:, 0, 1, :, :].astype(np.float64)      # (192, 3, 3)
    fc_w = inp['fc_w'][:, :, 0, 0, 0].astype(np.float64)        # (9, 24)
    fc_b = inp['fc_b'].astype(np.float32)
    dep_mid = inp['dep_w'][:, :, 1, :, :].astype(np.float64)    # (64, 9, 3, 3)
    dep_b = inp['dep_b'].astype(np.float32)
    ln_g = inp['ln_g'].astype(np.float64)
    ln_b = inp['ln_b'].astype(np.float64)
    A = -np.exp(inp['A_log'].astype(np.float64))                # (NG, DG, DSTATE)
    Wdt, bdt = inp['Wdt'].astype(np.float64), inp['bdt'].astype(np.float64)
    WB, WC = inp['WB'].astype(np.float64), inp['WC'].astype(np.float64)
    Dp = inp['Dp'].astype(np.float64)
    out_w, out_b = inp['out_w'].astype(np.float64), inp['out_b'].astype(np.float64)

    c = {}
    # conv branch: fused tap weights Wt(ty,tx) = FCbd . diag(dw_tap) . Wqkv
    FCbd = np.zeros((72, 192))
    for d in range(8):
        for o in range(9):
            for k in range(24):
                FCbd[d * 9 + o, k * 8 + d] = fc_w[o, k]
    Wt = {}
    for ty in range(3):
        for tx in range(3):
            Wt[(ty, tx)] = FCbd @ (dw_mid[:, ty, tx][:, None] * qkv_w)  # (72,64)
    # paired taps (ty,0)+(ty,1) share one 128-contraction matmul; x_pad rows
    # 64-127 hold the 1-column-shifted image.
    wpair = np.zeros((128, 3 * 72), np.float32)
    wsing = np.zeros((64, 3 * 72), np.float32)
    for ty in range(3):
        wpair[0:64, 72 * ty:72 * ty + 72] = Wt[(ty, 0)].T
        wpair[64:128, 72 * ty:72 * ty + 72] = Wt[(ty, 1)].T
        wsing[:, 72 * ty:72 * ty + 72] = Wt[(ty, 2)].T
    c['wpair'] = wpair
    c['wsing'] = wsing
    f2b = np.zeros((72, 1), np.float32)
    for d in range(8):
        for o in range(9):
            f2b[d * 9 + o, 0] = fc_b[o]
    c['f2_bias'] = f2b
    bdep = np.zeros((72, 9 * 64), np.float32)
    for ty in range(3):
        for tx in range(3):
            k = ty * 3 + tx
            Bt = np.zeros((64, 72))
            for g in range(8):
                Bt[8 * g:8 * g + 8, 9 * g:9 * g + 9] = dep_mid[8 * g:8 * g + 8, :, ty, tx]
            bdep[:, 64 * k:64 * k + 64] = Bt.T
    c['bdep'] = bdep
    c['depb_pp'] = np.tile(dep_b, IPC).reshape(128, 1)

    # mamba projections (gamma folded into rows; mu handled via explicit
    # xcen). B/C interleaved per group: bc rows g*16+0..8 = B_g, +8..16 = C_g
    # so each (image, group) block of bc_sb is 16 contiguous rows.
    dtbc = np.zeros((64, 128))
    for g in range(NG):
        rows = slice(g * DG, (g + 1) * DG)
        gam = ln_g[rows][:, None]
        dtbc[rows, g * DG:(g + 1) * DG] = Wdt[g] * gam
        dtbc[rows, 64 + g * 16: 64 + g * 16 + 8] = WB[g] * gam
        dtbc[rows, 64 + g * 16 + 8: 64 + g * 16 + 16] = WC[g] * gam
    # duplicated halves: image i's matmul uses rows i*64.. so lhsT base
    # partition matches its rhs (xcen rows i*64..)
    c['dtbc_lhsT'] = np.vstack([dtbc, dtbc])
    # dt bias: bdt + Wdt^T beta (per group)
    bdte = np.zeros((64, 1))
    for g in range(NG):
        bet = ln_b[g * DG:(g + 1) * DG]
        bdte[g * DG:(g + 1) * DG, 0] = bdt[g] + Wdt[g].T @ bet
    c['bdt_pp'] = bdte
    fbc = np.zeros((64, 1), np.float32)
    for g in range(NG):
        bet = ln_b[g * DG:(g + 1) * DG]
        fbc[g * 16: g * 16 + 8, 0] = (WB[g].T @ bet).astype(np.float32)
        fbc[g * 16 + 8: g * 16 + 16, 0] = (WC[g].T @ bet).astype(np.float32)
    c['fbc_pp'] = fbc
    c['beta_pp'] = np.tile(ln_b, IPC).reshape(128, 1)
    c['gamma_pp'] = np.tile(ln_g, IPC).reshape(128, 1)
    # scan lanes are d-major: p = d*8 + n (dt replicates with one inner-
    # broadcast DMA, B/C with plain-slice log-doubling)
    app = np.zeros((128, NG), np.float32)
    for g in range(NG):
        for n in range(DSTATE):
            for d in range(DG):
                app[d * 8 + n, g] = A[g, d, n]
    c['a_pp'] = app
    opl = np.zeros((128, NG * 64), np.float32)
    for g in range(NG):
        for n in range(DSTATE):
            for d in range(DG):
                opl[d * 8 + n, g * 64:(g + 1) * 64] = out_w[:, g * DG + d]
    c['outproj_lhsT'] = opl
    dpf = out_w * Dp.reshape(-1)[None, :]
    dpl = np.zeros((128, 128), np.float32)
    for i in range(IPC):
        dpl[i * 64:(i + 1) * 64, i * 64:(i + 1) * 64] = dpf.T
    c['dp_lhsT'] = dpl
    outb_eff = out_b + dpf @ ln_b
    c['outb_pp'] = np.tile(outb_eff, IPC).reshape(128, 1)
    c['ca1_lhsT'] = (inp['ca_w1'].T / L).astype(np.float32)     # fold 1/L mean
    c['ca1_b'] = inp['ca_b1'].reshape(16, 1).astype(np.float32)
    c['ca2_lhsT'] = inp['ca_w2'].T.astype(np.float32)
    c['ones64'] = np.ones((64, 1), np.float32)
    c['ca2bn_pp'] = -np.tile(inp['ca_b2'], IPC).reshape(128, 1)
    sl = np.zeros((128, 2), np.float32)
    sl[0:64, 0] = 1.0
    sl[64:128, 1] = 1.0
    c['stats_lhsT'] = sl
    return {k: np.ascontiguousarray(np.asarray(v, np.float32)) for k, v in c.items()}


CONST_SPECS = [
    ('wpair', [128, 3 * 72], BF16), ('wsing', [64, 3 * 72], BF16),
    ('f2_bias', [72, 1], F32),
    ('bdep', [72, 9 * 64], BF16), ('depb_pp', [128, 1], F32),
    ('dtbc_lhsT', [128, 128], BF16),
    ('bdt_pp', [64, 1], F32), ('fbc_pp', [64, 1], F32),
    ('beta_pp', [128, 1], F32), ('gamma_pp', [128, 1], F32),
    ('a_pp', [128, NG], F32),
    ('outproj_lhsT', [128, NG * 64], BF16), ('dp_lhsT', [128, 128], BF16),
    ('outb_pp', [128, 1], F32),
    ('ca1_lhsT', [64, 16], BF16), ('ca1_b', [16, 1], F32),
    ('ca2_lhsT', [16, 64], BF16), ('ca2bn_pp', [128, 1], F32),
    ('ones64', [64, 1], F32),
    ('stats_lhsT', [128, 2], BF16),
]


def _build(reps=1, beta_zero=False):
    _patch_bass_class()
    nc = bass.Bass("TRN2")
    xin = nc.declare_dram_parameter("x", [IPC, C, H, W], F32, isOutput=False)
    out = nc.declare_dram_parameter("out", [IPC, C, H, W], F32, isOutput=True)
    dram = {n: nc.declare_dram_parameter(n, s, F32, isOutput=False)
            for n, s, _ in CONST_SPECS}

    xin_f = xin.rearrange("i c h w -> (i c) (h w)")
    out_f = out.rearrange("i c h w -> (i c) (h w)")

    with TileContext(nc) as tc:
        with tc.tile_pool(name="const", bufs=1) as kpool, \
             tc.tile_pool(name="pers", bufs=1) as pp, \
             tc.tile_pool(name="work", bufs=2) as wp, \
             tc.tile_pool(name="ps", bufs=1, space="PSUM") as ps:

            kt = {}
            for name, shape, dt in CONST_SPECS:
                kt[name] = kpool.tile(shape, dt, tag=name, name=name)
                eng = nc.gpsimd if dt == BF16 else nc.sync
                eng.dma_start(kt[name][:], dram[name][:])

            # ---- persistent SBUF tiles (stable addresses across reps;
            # written in place every rep, tags never shared) ----
            xb = pp.tile([128, L], BF16, tag="xb", name="xb")
            x_pad = [pp.tile([128, PADL], BF16, tag=f"x_pad{i}", name=f"x_pad{i}")
                     for i in range(IPC)]
            f2_pad = [pp.tile([72, PADL], BF16, tag=f"f2_pad{i}", name=f"f2_pad{i}")
                      for i in range(IPC)]
            srow = pp.tile([2, L], BF16, tag="srow", name="srow")
            xcen = pp.tile([128, L], BF16, tag="xcen", name="xcen")
            xs = pp.tile([128, L], BF16, tag="xs", name="xs")
            dt_hat = pp.tile([128, L], BF16, tag="dt_hat", name="dt_hat")
            bc_sb = pp.tile([128, L], BF16, tag="bc_sb", name="bc_sb")
            u_sb = pp.tile([128, L], BF16, tag="u_sb", name="u_sb")
            xnc = pp.tile([128, L], BF16, tag="xnc", name="xnc")
            y_sb = pp.tile([128, L], BF16, tag="y_sb", name="y_sb")
            oc_sb = pp.tile([128, L], BF16, tag="oc_sb", name="oc_sb")
            ymean = pp.tile([128, 1], F32, tag="ymean", name="ymean")
            ca_sb = pp.tile([128, 1], F32, tag="ca_sb", name="ca_sb")
            # the whole PSUM (8 banks) as one tile; matmuls write bank-aligned
            # slices, ACT/DVE consume multi-bank regions in one op
            psA = ps.tile([128, L], F32, tag="psA", name="psA")

            # pad borders zeroed once; interiors rewritten every rep and the
            # pad slots are never reused, so borders stay zero.
            for i in range(IPC):
                nc.vector.memset(x_pad[i][:], 0.0)
                nc.vector.memset(f2_pad[i][:], 0.0)

            from contextlib import nullcontext
            with tc.For_i(0, reps) if reps > 1 else nullcontext():
                # ---- input loads: one casting DMA (gpsimd), pads derive
                # from xb on the otherwise-idle sync/scalar queues ----
                nc.gpsimd.dma_start(xb[:], xin_f[:, :])
                for i, eng in ((0, nc.sync), (1, nc.scalar)):
                    xpv = x_pad[i].rearrange("c (h w) -> c h w", h=Hp)
                    src = xb[i * 64:(i + 1) * 64, :].rearrange(
                        "c (h w) -> c h w", h=H)
                    eng.dma_start(xpv[0:64, 1:H + 1, 1:W + 1], src)
                    eng.dma_start(xpv[64:128, 1:H + 1, 0:W], src)

                # ---- conv1: f2 = sum_tap Wt_tap @ x_shift_tap + fc_b ----
                # tap-outer, chunk-inner: consecutive matmuls share weights so
                # _dedup_ldweights_json keeps one PE array load per tap
                for i in range(IPC):
                    xpv = x_pad[i].rearrange("c (h w) -> c h w", h=Hp)
                    f2v = f2_pad[i].rearrange("c (h w) -> c h w", h=Hp)
                    for ty in range(3):
                        for cb in range(NCH):
                            nc.tensor.matmul(
                                psA[0:72, cb * TC:(cb + 1) * TC],
                                kt['wpair'][:, 72 * ty:72 * ty + 72],
                                xpv[:, 8 * cb + ty: 8 * cb + ty + 8, 0:64],
                                start=(ty == 0), stop=False,
                                skip_group_check=True)
                    for ty in range(3):
                        for cb in range(NCH):
                            nc.tensor.matmul(
                                psA[0:72, cb * TC:(cb + 1) * TC],
                                kt['wsing'][:, 72 * ty:72 * ty + 72],
                                xpv[0:64, 8 * cb + ty: 8 * cb + ty + 8, 2:66],
                                start=False, stop=(ty == 2),
                                skip_group_check=True)
                    nc.scalar.activation(
                        f2v[:, 1:H + 1, 1:W + 1],
                        psA[0:72, :].rearrange("c (a b) -> c a b", a=H),
                        AF.Identity, bias=kt['f2_bias'][:])

                # ---- stats: per-pixel mean/rstd over channels ----
                sqf = pp.tile([128, L], BF16, tag="dAc", name="sqf")
                nc.vector.tensor_mul(sqf[:], xb[:], xb[:])
                for cb in range(NCH):
                    cs = slice(cb * TC, (cb + 1) * TC)
                    nc.tensor.matmul(psA[0:2, cs], kt['stats_lhsT'][:],
                                     xb[:, cs], start=True, stop=True,
                                     skip_group_check=True)
                for cb in range(NCH):
                    cs = slice(cb * TC, (cb + 1) * TC)
                    nc.tensor.matmul(psA[32:34, cs], kt['stats_lhsT'][:],
                                     sqf[:, cs], start=True, stop=True,
                                     tile_position=(0, 32),
                                     skip_group_check=True)
                nc.scalar.activation(srow[0:2, :], psA[0:2, :], AF.Identity,
                                     scale=1.0 / 64)
                nc.scalar.activation(xcen[0:2, :], psA[32:34, :], AF.Identity,
                                     scale=1.0 / 64)
                # mr: cols 0:L = per-row mu, cols L:2L = per-row rstd;
                # replicated by log-doubling (stride-0 broadcast DMAs are
                # ~16x slower per row than real copies)
                mr = pp.tile([128, 2 * L], BF16, tag="bc_rep", name="mr",
                             bufs=2)
                for i in range(IPC):
                    nc.scalar.dma_start(mr[i * 64:i * 64 + 1, 0:L],
                                        srow[i:i + 1, :])
                nc.vector.tensor_mul(srow[0:2, :], srow[0:2, :], srow[0:2, :])
                nc.vector.tensor_sub(xcen[0:2, :], xcen[0:2, :], srow[0:2, :])
                # (the reference's +1e-5 eps is below bf16 resolution at var~1)
                nc.scalar.activation(xcen[0:2, :], xcen[0:2, :], AF.Ln)
                nc.scalar.activation(xcen[0:2, :], xcen[0:2, :], AF.Exp,
                                     scale=-0.5)
                for i in range(IPC):
                    nc.scalar.dma_start(mr[i * 64:i * 64 + 1, L:2 * L],
                                        xcen[i:i + 1, :])
                for k in (1, 2, 4, 8, 16, 32):
                    for i in range(IPC):
                        nc.sync.dma_start(
                            mr[i * 64 + k: i * 64 + 2 * k, 0:L],
                            mr[i * 64: i * 64 + k, 0:L])
                        nc.scalar.dma_start(
                            mr[i * 64 + k: i * 64 + 2 * k, L:2 * L],
                            mr[i * 64: i * 64 + k, L:2 * L])
                musb = mr[:, 0:L]
                rstb = mr[:, L:2 * L]
                nc.vector.tensor_sub(xcen[:], xb[:], musb[:])
                # xs = rstd * (x - mu): fold the per-pixel rstd into the
                # projection rhs (linear), so no 128-row rstd broadcast needed
                nc.vector.tensor_mul(xs[:], xcen[:], rstb[:])

                # ---- dt/B/C projections (rhs pre-scaled by rstd) ----
                for i in range(IPC):
                    for cb in range(NCH):
                        cs = slice(cb * TC, (cb + 1) * TC)
                        nc.tensor.matmul(psA[:, cs],
                                         kt['dtbc_lhsT'][i * 64:(i + 1) * 64, :],
                                         xs[i * 64:(i + 1) * 64, cs],
                                         start=True, stop=True,
                                         skip_group_check=True)
                    et = pp.tile([128, L], BF16, tag="h_sb", name=f"et{i}")
                    nc.scalar.activation(et[0:64, :], psA[0:64, :], AF.Exp,
                                         bias=kt['bdt_pp'][:])
                    nc.scalar.activation(dt_hat[i * 64:(i + 1) * 64, :],
                                         et[0:64, :], AF.Ln,
                                         bias=kt['ones64'][:])
                    nc.scalar.activation(bc_sb[i * 64:(i + 1) * 64, :],
                                         psA[64:128, :], AF.Identity,
                                         bias=kt['fbc_pp'][:])
                # xn = gamma * xs ; u = dt * (xn + beta)
                nc.vector.tensor_scalar_mul(xnc[:], xs[:], kt['gamma_pp'][:])
                nc.vector.tensor_mul(u_sb[:], dt_hat[:], xnc[:])
                if not beta_zero:
                    nc.vector.scalar_tensor_tensor(u_sb[:], dt_hat[:],
                                                   kt['beta_pp'][:],
                                                   u_sb[:], OP.mult, OP.add)

                # ---- conv2: out_conv = sum_tap BDdep_tap @ f2_shift_tap ----
                # tap-outer, chunk-inner (one weight load per tap per image)
                for i in range(IPC):
                    f2v = f2_pad[i].rearrange("c (h w) -> c h w", h=Hp)
                    for ty in range(3):
                        for tx in range(3):
                            k = ty * 3 + tx
                            for cb in range(NCH):
                                nc.tensor.matmul(
                                    psA[i * 64:(i + 1) * 64,
                                        cb * TC:(cb + 1) * TC],
                                    kt['bdep'][:, 64 * k:64 * k + 64],
                                    f2v[:, 8 * cb + ty: 8 * cb + ty + 8, tx: tx + 64],
                                    start=(k == 0), stop=(k == 8),
                                    tile_position=(0, i * 64),
                                    skip_group_check=True)
                nc.scalar.activation(oc_sb[:], psA[:], AF.Identity,
                                     bias=kt['depb_pp'][:])

                # ---- selective scans; out-proj accumulates across all of
                # psA (start on g==0 per image half, stop on the dp matmul) --
                for i in range(IPC):
                    for g in range(NG):
                        colmajor = g >= 2
                        rev = (g % 2 == 1)
                        rs = slice(i * 64 + g * 16, i * 64 + (g + 1) * 16)
                        # d-major lanes: dt/u replicate 8x consecutive via
                        # one inner-broadcast DMA each (parallel queues)
                        du = pp.tile([128, 2 * L], BF16, tag="du", name="du",
                                     bufs=2)
                        nc.sync.dma_start(
                            du[:, 0:L],
                            dt_hat[rs, :].unsqueeze(1).broadcast_to([16, 8, L]))
                        nc.gpsimd.dma_start(
                            du[:, L:2 * L],
                            u_sb[rs, :].unsqueeze(1).broadcast_to([16, 8, L]))
                        dt_rep = du[:, 0:L]
                        u_rep = du[:, L:2 * L]
                        # B/C (lane index n = p%8): plain-slice log-doubling
                        bc_rep = pp.tile([128, 2 * L], BF16, tag="bc_rep",
                                         name="bc_rep", bufs=2)
                        base = i * 64 + g * 16
                        nc.scalar.dma_start(bc_rep[0:8, 0:L],
                                            bc_sb[base: base + 8, :])
                        nc.scalar.dma_start(bc_rep[0:8, L:2 * L],
                                            bc_sb[base + 8: base + 16, :])
                        nc.scalar.dma_start(bc_rep[8:16, :], bc_rep[0:8, :])
                        nc.scalar.dma_start(bc_rep[16:32, :], bc_rep[0:16, :])
                        nc.scalar.dma_start(bc_rep[32:64, :], bc_rep[0:32, :])
                        nc.scalar.dma_start(bc_rep[64:128, :], bc_rep[0:64, :])
                        b_rep = bc_rep[:, 0:L]
                        c_rep = bc_rep[:, L:2 * L]
                        h_sb = pp.tile([128, L], BF16, tag="h_sb", name="h_sb")
                        # dA/dBx; column-major groups pre-transpose into
                        # dedicated slots (scan operands must be 2D)
                        if colmajor:
                            dA = pp.tile([128, L], BF16, tag="dAc", name="dA")
                            dBx = pp.tile([128, L], BF16, tag="dBc", name="dBx")
                            nc.scalar.activation(
                                dA.rearrange("p (x y) -> p y x", x=W),
                                dt_rep.rearrange("p (y x) -> p y x", y=H),
                                AF.Exp, scale=kt['a_pp'][:, g:g + 1])
                            nc.vector.tensor_tensor(
                                dBx.rearrange("p (x y) -> p y x", x=W),
                                u_rep.rearrange("p (y x) -> p y x", y=H),
                                b_rep.rearrange("p (y x) -> p y x", y=H),
                                OP.mult)
                        else:
                            dA, dBx = dt_rep, u_rep
                            nc.scalar.activation(dA[:], dt_rep[:], AF.Exp,
                                                 scale=kt['a_pp'][:, g:g + 1])
                            nc.vector.tensor_mul(dBx[:], u_rep[:], b_rep[:])
                        if rev:
                            nc.vector.tensor_tensor_scan(
                                h_sb[:, ::-1], dA[:, ::-1], dBx[:, ::-1], 0.0,
                                OP.mult, OP.add)
                        else:
                            nc.vector.tensor_tensor_scan(
                                h_sb[:], dA[:], dBx[:], 0.0, OP.mult, OP.add)
                        z = pp.tile([128, L], BF16, tag="du", name="z",
                                    bufs=2)
                        if colmajor:
                            nc.vector.tensor_tensor(
                                z.rearrange("p (y x) -> p y x", y=H),
                                h_sb.rearrange("p (x y) -> p y x", x=W),
                                c_rep.rearrange("p (y x) -> p y x", y=H),
                                OP.mult)
                        else:
                            nc.vector.tensor_mul(z[:], h_sb[:], c_rep[:])
                        for cb in range(NCH):
                            cs = slice(cb * TC, (cb + 1) * TC)
                            nc.tensor.matmul(
                                psA[i * 64:(i + 1) * 64, cs],
                                kt['outproj_lhsT'][:, g * 64:(g + 1) * 64],
                                z[:, cs], start=(g == 0), stop=False,
                                tile_position=(0, i * 64),
                                skip_group_check=True)
                for cb in range(NCH):
                    cs = slice(cb * TC, (cb + 1) * TC)
                    nc.tensor.matmul(psA[:, cs], kt['dp_lhsT'][:], xnc[:, cs],
                                     start=False, stop=True,
                                     skip_group_check=True)
                nc.scalar.activation(y_sb[:], psA[:], AF.Identity,
                                     bias=kt['outb_pp'][:],
                                     accum_out=ymean[:, 0:1])

                # ---- CA gate ----
                ymc = []
                for i in range(IPC):
                    t = wp.tile([64, 1], BF16, tag=f"ymc{i}", name=f"ymc{i}")
                    nc.gpsimd.dma_start(t[:], ymean[i * 64:(i + 1) * 64, :])
                    ymc.append(t)
                for i in range(IPC):
                    nc.tensor.matmul(psA[0:16, i:i + 1], kt['ca1_lhsT'][:],
                                     ymc[i][:], start=True, stop=True,
                                     skip_group_check=True)
                ca1s = wp.tile([16, IPC], BF16, tag="ca1s", name="ca1s")
                nc.scalar.activation(ca1s[:], psA[0:16, 0:IPC], AF.Relu,
                                     bias=kt['ca1_b'][:])
                for i in range(IPC):
                    nc.tensor.matmul(psA[i * 64:(i + 1) * 64, 4:5],
                                     kt['ca2_lhsT'][:],
                                     ca1s[:, i:i + 1], start=True, stop=True,
                                     tile_position=(0, i * 64),
                                     skip_group_check=True)
                nc.scalar.activation(ca_sb[:], psA[:, 4:5], AF.Exp, scale=-1.0,
                                     bias=kt['ca2bn_pp'][:])
                nc.vector.tensor_scalar_add(ca_sb[:], ca_sb[:], 1.0)
                nc.vector.reciprocal(ca_sb[:], ca_sb[:])

                # ---- final combine: out = x + oc(+depb) + ca*y ----
                # (u_sb and xcen are dead by now; reuse their APs in place)
                nc.vector.tensor_add(u_sb[:], oc_sb[:], xb[:])
                nc.vector.scalar_tensor_tensor(xcen[:], y_sb[:], ca_sb[:],
                                               u_sb[:], OP.mult, OP.add)
                nc.gpsimd.dma_start(out_f[:, :], xcen[:])

    # extended/loop InstISA instructions need .instr bytes populated before
    # the NEFF compiler sees them ("ISA wrong length" otherwise)
    mybir.codegen_inst_isa_subclasses(nc)
    return nc


def kernel(__reps=1, **inputs):
    inputs = {k: np.asarray(v) for k, v in inputs.items()}
    x = inputs['x'].astype(np.float32)
    beta_zero = bool(np.all(np.asarray(inputs['ln_b']) == 0))
    key = f"v2r{__reps}b{int(beta_zero)}"
    if key not in _CACHE:
        _CACHE[key] = _build(__reps, beta_zero)
    nc = _CACHE[key]
    consts = _make_consts(inputs)
    in_maps = []
    for core in range(NCORES):
        m = {'x': np.ascontiguousarray(x[core * IPC:(core + 1) * IPC])}
        m.update(consts)
        in_maps.append(m)
    res = run_bass_kernel_spmd(nc, in_maps, list(range(NCORES)))
    outs = [res.results[i]['out'] for i in range(NCORES)]
    return np.concatenate(outs, axis=0).astype(np.float32)
